# revision 37
# baseline (speedup 1.0000x reference)
"""Bass/Tile TRN2 kernel for nn_AttentionHead: single-head attention with
q/k/v projections (512->64), key mask, softmax over 4096 keys.

Sharding: 8 cores; core c handles batch c//2, query-half c%2 (2048 queries),
with that batch's full k/v replicated. No collectives.

Per-core dataflow (two stages; k/v streaming overlaps attention compute):
  - SWDGE cast-DMA loads q/k/v fp32 -> bf16 staged [t, d] tiles
  - PE transposes 128x128 blocks -> qT/kT/vT in [d, t] layout
  - TensorE projections: QT/KT [e, t] (duplicated on partitions 64-127 for
    row-packed scores); V via V^T then PE transpose -> V1 [t2, 65] where
    column 64 holds the key mask and V rows are pre-multiplied by the mask
    (masked softmax == sum(mask*exp*V) / sum(mask*exp), no -1e9 bias needed)
  - scores: S^T chunks [t2=128, t1=512] = KT_chunk.T @ QT (contract e=64);
    chunk pairs run concurrently in array row groups 0-63/64-127
  - ScalarE: exp(0.125 * S^T), one call per [128, 1024] psum pair
  - PV: O^T[65, t1] += V1_chunk.T @ expS (row 64 = denominator); PV matmuls
    are emitted one quad behind the scores so the in-order PE never stalls
  - epilogue: PE transpose [65,128] blocks, reciprocal + scale on VectorE
"""

import sys
import types

import numpy as np

import concourse.bass as bass
import concourse.tile as tile
from concourse import bacc, mybir
from concourse.masks import make_identity

B, T1, T2, D, E = 4, 4096, 4096, 512, 64
P = 128
F32 = mybir.dt.float32
BF16 = mybir.dt.bfloat16
EXPF = mybir.ActivationFunctionType.Exp
MULT = mybir.AluOpType.mult
ADD = mybir.AluOpType.add


def _install_ntff_hook():
    """Make trace=True usable under axon when antenv.axon_hooks is absent."""
    try:
        import antenv.axon_hooks  # noqa: F401
        return
    except ImportError:
        pass
    try:
        from trn_agent_boot.trn_boot import _ntff_profile_via_ctypes
        hook = _ntff_profile_via_ctypes("/opt/axon/libaxon_pjrt.so")
    except Exception:
        hook = None
    mod = types.ModuleType("antenv.axon_hooks")
    mod.get_axon_ntff_profile_hook = lambda: hook
    mod.set_axon_ntff_profile_hook = lambda h: None
    sys.modules["antenv.axon_hooks"] = mod


def _bcast_ap(ap, parts):
    """Broadcast a 1-D DRAM AP across `parts` partitions (stride-0 DMA)."""
    return bass.AP(tensor=ap.tensor, offset=ap.offset, ap=[[0, parts], ap.ap[0]])


def build_body(tc, nc, q, k, v, mask, Wq, bq, Wk, bk, Wv, bv, out, t1l, t2):
    DC = D // P            # 4 d-chunks
    NT2 = t2 // P          # t2 chunks of 128
    NT1 = t1l // P
    TB = 512               # staging/projection block (t rows)
    T1B = min(1024, t1l)   # phase-B t1 pass width

    with (
        tc.tile_pool(name="consts", bufs=1) as consts,
        tc.tile_pool(name="persist", bufs=1) as persist,
    ):
        ident_b = consts.tile([P, P], BF16)
        make_identity(nc, ident_b)
        ident_f = consts.tile([P, P], F32)
        make_identity(nc, ident_f)

        # weights, bf16, d on partitions: [P, DC, E]
        wq_b = consts.tile([P, DC, E], BF16)
        nc.gpsimd.dma_start(out=wq_b, in_=Wq.rearrange("(c p) e -> p c e", p=P))
        wk_b = consts.tile([P, DC, E], BF16)
        nc.gpsimd.dma_start(out=wk_b, in_=Wk.rearrange("(c p) e -> p c e", p=P))
        wv_b = consts.tile([P, DC, E], BF16)
        nc.gpsimd.dma_start(out=wv_b, in_=Wv.rearrange("(c p) e -> p c e", p=P))

        # biases: per-partition [E, 1] for QT/KT evac; broadcast [P, E] for V
        bq_s = consts.tile([E, 1], F32)
        nc.sync.dma_start(out=bq_s, in_=bq[:, None])
        bk_s = consts.tile([E, 1], F32)
        nc.sync.dma_start(out=bk_s, in_=bk[:, None])
        bv_s = consts.tile([E, 1], F32)
        nc.sync.dma_start(out=bv_s, in_=bv[:, None])

        # mask values per key, [partition = t2 % 128, col = t2 // 128]
        mk = consts.tile([P, NT2], F32)
        nc.sync.dma_start(out=mk, in_=mask.rearrange("(c p) -> p c", p=P))

        qT = persist.tile([P, DC, t1l], BF16)
        kT = persist.tile([P, DC, t2], BF16)
        vT = persist.tile([P, DC, t2], BF16)
        # QT/KT duplicated on partitions 64-127 for row-packed score matmuls
        QT = persist.tile([P, t1l], BF16)
        KT = persist.tile([P, t2], BF16)
        V1 = persist.tile([P, NT2, E + 1], BF16)
        out_sb = persist.tile([P, NT1, E], F32)

        # the "ones" column of V1 carries the mask directly: the masked
        # softmax denominator is sum(mask * exp)
        nc.vector.tensor_copy(out=V1[:, :, E], in_=mk)

        # Stage 1 streams k/v 512-row blocks (load -> transpose -> project)
        # and runs scores/exp/PV for the first two t1-halves on each chunk
        # pair as it becomes ready. Stage 2 finishes the remaining t1-halves
        # from SBUF-resident KT/V1. Score matmul pairs run concurrently in
        # array rows 0-63 / 64-127 (row packing, contract dim is 64) and
        # share one exp call; the mask is folded into V1 rows.
        HW = min(512, t1l)
        NHALF = t1l // HW
        stream_halves = list(range(min(2, NHALF)))
        post_halves = list(range(len(stream_halves), NHALF))
        NP2 = max(1, NT2 // 2)
        pv_tiles = {}
        evac_flip = [0]

        with (
            tc.tile_pool(name="expp", bufs=4) as expp,
            tc.tile_pool(name="ep", bufs=3) as ep,
            tc.tile_pool(name="psPV", bufs=1, space="PSUM") as psPV,
        ):
            def evac_copy(out_ap, in_ap):
                # 2-of-3 on DVE, 1-of-3 on ACT (ACT also carries the exps)
                evac_flip[0] = (evac_flip[0] + 1) % 3
                if evac_flip[0]:
                    nc.vector.tensor_copy(out=out_ap, in_=in_ap)
                else:
                    nc.scalar.copy(out=out_ap, in_=in_ap)

            # software pipeline: PV matmuls for a (quad, half) are emitted
            # only after the NEXT quad's score matmuls, so the in-order PE
            # stream never stalls waiting for the current exp. Scores land
            # in bf16 PSUM: a 4-chunk quad fits 2 banks and one exp call.
            pending = []
            CPQ = min(2, NT2)   # chunks per score-psum tile
            NQ = max(1, NT2 // CPQ)
            QW = CPQ * HW

            def emit_pv(item):
                qi, h, ex = item
                for u in range(CPQ):
                    c = CPQ * qi + u
                    nc.tensor.matmul(
                        pv_tiles[h], V1[:, c, :], ex[:, u * HW:(u + 1) * HW],
                        start=(c == 0), stop=(c == NT2 - 1))

            def scores_exp_pv(psS, qi, h, tag):
                q0 = h * HW
                ps = psS.tile([P, QW], F32, tag=tag, name=f"s_{h}_{qi}")
                for u in range(CPQ):
                    c = CPQ * qi + u
                    rg = E * (u % 2)
                    nc.tensor.matmul(
                        ps[:, u * HW:(u + 1) * HW],
                        KT[rg:rg + E, c * P:(c + 1) * P],
                        QT[rg:rg + E, q0:q0 + HW], start=True, stop=True,
                        tile_position=(rg, 0))
                ex = expp.tile([P, QW], BF16, tag="e", name=f"e_{h}_{qi}")
                nc.scalar.activation(out=ex, in_=ps, func=EXPF, scale=0.125)
                pending.append((qi, h, ex))
                while len(pending) > 1:
                    emit_pv(pending.pop(0))

            def flush_pv():
                while pending:
                    emit_pv(pending.pop(0))

            orr = out.rearrange("(n p) e -> p n e", p=P)

            def epilogue(h, psO):
                pvt = pv_tiles.pop(h)
                q0 = h * HW
                n0, n1 = q0 // P, (q0 + HW) // P
                ov = ep.tile([E + 1, HW], F32, tag="ov", name=f"ov_{h}")
                nc.vector.tensor_copy(out=ov, in_=pvt)
                for j in range(HW // P):
                    po = psO.tile([P, E + 1], F32, tag="o",
                                  name=f"o_{h}_{j}")
                    nc.tensor.transpose(
                        po, ov[:, j * P:(j + 1) * P],
                        ident_f[0:E + 1, 0:E + 1])
                    rec = ep.tile([P, 1], F32, tag="rec",
                                  name=f"rec_{h}_{j}")
                    nc.vector.reciprocal(rec, po[:, E:E + 1])
                    nc.vector.tensor_scalar_mul(
                        out_sb[:, (q0 + j * P) // P, :], po[:, 0:E], rec)
                nc.sync.dma_start(out=orr[:, n0:n1, :],
                                  in_=out_sb[:, n0:n1, :])

            # ---------------- stage 1: stream ----------------
            with (
                tc.tile_pool(name="stage", bufs=8) as stagep,
                tc.tile_pool(name="psA", bufs=2, space="PSUM") as psA,
                tc.tile_pool(name="psAp", bufs=1, space="PSUM") as psAp,
                tc.tile_pool(name="psS1", bufs=1, space="PSUM") as psS1,
            ):
                for h in stream_halves:
                    pv_tiles[h] = psPV.tile([E + 1, HW], F32,
                                            tag=f"pv{h % 2}", name=f"pv_{h}")

                def lt_block(srcr, dst_T, tb, nsub):
                    st = stagep.tile([P, nsub, D], BF16, tag="stage",
                                     name=f"st_{dst_T.tensor.name}_{tb}")
                    nc.gpsimd.dma_start(
                        out=st, in_=srcr[:, tb * nsub:(tb + 1) * nsub, :])
                    grp = 2 if nsub % 2 == 0 else 1
                    for ns0 in range(0, nsub, grp):
                        pst = psA.tile([P, grp * D], BF16, tag="tps",
                                       name=f"tps_{tb}_{ns0}")
                        for g in range(grp):
                            for j in range(DC):
                                nc.tensor.transpose(
                                    pst[:, g * D + j * P:g * D + (j + 1) * P],
                                    st[:, ns0 + g, j * P:(j + 1) * P],
                                    ident_b)
                        t0 = tb * nsub * P + ns0 * P
                        evac_copy(
                            dst_T[:, :, t0:t0 + grp * P].rearrange(
                                "p j (g c) -> p g j c", c=P),
                            pst.rearrange("p (g j c) -> p g j c", j=DC, c=P))

                def proj_block(src_T, w_b, b_s, dst, tb, tb_sz):
                    # write projection to partitions 0:64 and dup to 64:128
                    ps = psAp.tile([E, tb_sz], F32, tag="pproj",
                                   name=f"pp_{dst.tensor.name}_{tb}")
                    for j in range(DC):
                        nc.tensor.matmul(
                            ps, w_b[:, j],
                            src_T[:, j, tb * tb_sz:(tb + 1) * tb_sz],
                            start=(j == 0), stop=(j == DC - 1))
                    sl = slice(tb * tb_sz, (tb + 1) * tb_sz)
                    nc.vector.tensor_scalar_add(dst[0:E, sl], ps, b_s)
                    nc.scalar.activation(
                        out=dst[E:2 * E, sl], in_=ps,
                        func=mybir.ActivationFunctionType.Identity,
                        bias=b_s, scale=1.0)

                # q pipeline (small staging blocks: faster PE rampup)
                TBQ = min(256, t1l)
                qsr = q.rearrange("(n p) d -> p n d", p=P)
                for tb in range(t1l // TBQ):
                    lt_block(qsr, qT, tb, TBQ // P)
                for tb in range(t1l // TBQ):
                    proj_block(qT, wq_b, bq_s, QT, tb, TBQ)

                # k/v stream; per 512-row block: 4 chunks = 2 score pairs
                TBK = min(TB, t2)
                ksr = k.rearrange("(n p) d -> p n d", p=P)
                vsr = v.rearrange("(n p) d -> p n d", p=P)
                cpb = TBK // P           # chunks per block (= one quad)
                for blk in range(t2 // TBK):
                    lt_block(ksr, kT, blk, cpb)
                    proj_block(kT, wk_b, bk_s, KT, blk, TBK)
                    lt_block(vsr, vT, blk, cpb)
                    # V^T block [E, TBK] like K^T, then PE-transpose each
                    # 128-chunk to natural layout, masking on evacuation
                    psv = psAp.tile([E, TBK], F32, tag="psv",
                                    name=f"psv_{blk}")
                    for j in range(DC):
                        nc.tensor.matmul(
                            psv, wv_b[:, j],
                            vT[:, j, blk * TBK:(blk + 1) * TBK],
                            start=(j == 0), stop=(j == DC - 1))
                    vts = ep.tile([E, TBK], BF16, tag="vts",
                                  name=f"vts_{blk}")
                    nc.vector.tensor_scalar_add(vts, psv, bv_s)
                    for ci in range(cpb):
                        c = blk * cpb + ci
                        pvn = psA.tile([P, E], BF16, tag="tps",
                                       name=f"pvn_{c}")
                        nc.tensor.transpose(
                            pvn, vts[:, ci * P:(ci + 1) * P],
                            ident_b[0:E, 0:E])
                        # fold the key mask into V rows: masked softmax
                        # = sum(mask*exp*V) / sum(mask*exp)
                        nc.vector.tensor_scalar_mul(
                            V1[:, c, 0:E], pvn, mk[:, c:c + 1])
                    for qb in range(max(1, cpb // CPQ)):
                        for h in stream_halves:
                            scores_exp_pv(psS1, blk * max(1, cpb // CPQ) + qb,
                                          h, "s1")

            # ---------------- stage 2: remaining t1-halves ----------------
            with (
                tc.tile_pool(name="psS2", bufs=2, space="PSUM") as psS2,
                tc.tile_pool(name="psO", bufs=2, space="PSUM") as psO,
            ):
                flush_pv()
                pending_stream_epi = list(stream_halves)
                if post_halves and pending_stream_epi:
                    # free pv0 so the first post half can start accumulating
                    epilogue(pending_stream_epi.pop(0), psO)

                def drain_stream_epi():
                    while pending_stream_epi:
                        epilogue(pending_stream_epi.pop(0), psO)

                if not post_halves:
                    drain_stream_epi()
                for h in post_halves:
                    pv_tiles[h] = psPV.tile([E + 1, HW], F32,
                                            tag=f"pv{h % 2}", name=f"pv_{h}")
                    for qi in range(NQ):
                        scores_exp_pv(psS2, qi, h, "s2")
                        if qi >= 1:
                            drain_stream_epi()
                    drain_stream_epi()
                    flush_pv()
                    epilogue(h, psO)


def build_nc(t1l=T1 // 2, t2=T2):
    nc = bacc.Bacc()
    q = nc.declare_dram_parameter("q", [t1l, D], F32, isOutput=False)
    k = nc.declare_dram_parameter("k", [t2, D], F32, isOutput=False)
    v = nc.declare_dram_parameter("v", [t2, D], F32, isOutput=False)
    mask = nc.declare_dram_parameter("mask", [t2], F32, isOutput=False)
    Wq = nc.declare_dram_parameter("Wq", [D, E], F32, isOutput=False)
    bq = nc.declare_dram_parameter("bq", [E], F32, isOutput=False)
    Wk = nc.declare_dram_parameter("Wk", [D, E], F32, isOutput=False)
    bk = nc.declare_dram_parameter("bk", [E], F32, isOutput=False)
    Wv = nc.declare_dram_parameter("Wv", [D, E], F32, isOutput=False)
    bv = nc.declare_dram_parameter("bv", [E], F32, isOutput=False)
    out = nc.declare_dram_parameter("out", [t1l, E], F32, isOutput=True)
    with tile.TileContext(nc) as tc:
        build_body(tc, nc, q[:], k[:], v[:], mask[:], Wq[:], bq[:], Wk[:],
                   bk[:], Wv[:], bv[:], out[:], t1l, t2)
    nc.compile()
    return nc


_NC_CACHE = {}


def _get_nc():
    if "nc" not in _NC_CACHE:
        _NC_CACHE["nc"] = build_nc()
    return _NC_CACHE["nc"]


def make_in_maps(q, k, v, mask, Wq, bq, Wk, bk, Wv, bv):
    t1l = T1 // 2
    shared = {
        "Wq": np.ascontiguousarray(Wq, np.float32),
        "bq": np.ascontiguousarray(bq, np.float32),
        "Wk": np.ascontiguousarray(Wk, np.float32),
        "bk": np.ascontiguousarray(bk, np.float32),
        "Wv": np.ascontiguousarray(Wv, np.float32),
        "bv": np.ascontiguousarray(bv, np.float32),
    }
    in_maps = []
    for c in range(8):
        b, h = divmod(c, 2)
        in_maps.append({
            "q": np.ascontiguousarray(q[b, h * t1l:(h + 1) * t1l], np.float32),
            "k": np.ascontiguousarray(k[b], np.float32),
            "v": np.ascontiguousarray(v[b], np.float32),
            "mask": np.ascontiguousarray(mask[b, 0], np.float32),
            **shared,
        })
    return in_maps


def assemble_out(results):
    t1l = T1 // 2
    out = np.empty((B, T1, E), np.float32)
    for c in range(8):
        b, h = divmod(c, 2)
        out[b, h * t1l:(h + 1) * t1l] = results[c]["out"]
    return out


def run(inputs, trace=False):
    from concourse.bass_utils import run_bass_kernel_spmd
    _install_ntff_hook()
    nc = _get_nc()
    in_maps = make_in_maps(**inputs)
    res = run_bass_kernel_spmd(nc, in_maps, list(range(8)), trace=trace)
    return assemble_out(res.results), res


def kernel(q, k, v, mask, Wq, bq, Wk, bk, Wv, bv):
    out, _ = run(dict(q=q, k=k, v=v, mask=mask, Wq=Wq, bq=bq, Wk=Wk, bk=bk,
                      Wv=Wv, bv=bv))
    return out


# revision 39
# speedup vs baseline: 1.0239x; 1.0239x over previous
"""Bass/Tile TRN2 kernel for nn_AttentionHead: single-head attention with
q/k/v projections (512->64), key mask, softmax over 4096 keys.

Sharding: 8 cores; core c handles batch c//2, query-half c%2 (2048 queries),
with that batch's full k/v replicated. No collectives.

Per-core dataflow (two stages; k/v streaming overlaps attention compute):
  - SWDGE cast-DMA loads q/k/v fp32 -> bf16 staged [t, d] tiles
  - PE transposes 128x128 blocks -> qT/kT/vT in [d, t] layout
  - TensorE projections: QT/KT [e, t] (duplicated on partitions 64-127 for
    row-packed scores); V via V^T then PE transpose -> V1 [t2, 65] where
    column 64 holds the key mask and V rows are pre-multiplied by the mask
    (masked softmax == sum(mask*exp*V) / sum(mask*exp), no -1e9 bias needed)
  - scores: S^T chunks [t2=128, t1=512] = KT_chunk.T @ QT (contract e=64);
    chunk pairs run concurrently in array row groups 0-63/64-127
  - ScalarE: exp(0.125 * S^T), one call per [128, 1024] psum pair
  - PV: O^T[65, t1] += V1_chunk.T @ expS (row 64 = denominator); PV matmuls
    are emitted one quad behind the scores so the in-order PE never stalls
  - epilogue: PE transpose [65,128] blocks, reciprocal + scale on VectorE
"""

import sys
import types

import numpy as np

import concourse.bass as bass
import concourse.tile as tile
from concourse import bacc, mybir
from concourse.masks import make_identity

B, T1, T2, D, E = 4, 4096, 4096, 512, 64
P = 128
F32 = mybir.dt.float32
BF16 = mybir.dt.bfloat16
EXPF = mybir.ActivationFunctionType.Exp
MULT = mybir.AluOpType.mult
ADD = mybir.AluOpType.add


def _install_ntff_hook():
    """Make trace=True usable under axon when antenv.axon_hooks is absent."""
    try:
        import antenv.axon_hooks  # noqa: F401
        return
    except ImportError:
        pass
    try:
        from trn_agent_boot.trn_boot import _ntff_profile_via_ctypes
        hook = _ntff_profile_via_ctypes("/opt/axon/libaxon_pjrt.so")
    except Exception:
        hook = None
    mod = types.ModuleType("antenv.axon_hooks")
    mod.get_axon_ntff_profile_hook = lambda: hook
    mod.set_axon_ntff_profile_hook = lambda h: None
    sys.modules["antenv.axon_hooks"] = mod


def _bcast_ap(ap, parts):
    """Broadcast a 1-D DRAM AP across `parts` partitions (stride-0 DMA)."""
    return bass.AP(tensor=ap.tensor, offset=ap.offset, ap=[[0, parts], ap.ap[0]])


def build_body(tc, nc, q, k, v, mask, Wq, bq, Wk, bk, Wv, bv, out, t1l, t2):
    DC = D // P            # 4 d-chunks
    NT2 = t2 // P          # t2 chunks of 128
    NT1 = t1l // P
    TB = 512               # staging/projection block (t rows)
    T1B = min(1024, t1l)   # phase-B t1 pass width

    with (
        tc.tile_pool(name="consts", bufs=1) as consts,
        tc.tile_pool(name="persist", bufs=1) as persist,
    ):
        ident_b = consts.tile([P, P], BF16)
        make_identity(nc, ident_b)
        ident_f = consts.tile([P, P], F32)
        make_identity(nc, ident_f)

        # weights, bf16, d on partitions: [P, DC, E]
        wq_b = consts.tile([P, DC, E], BF16)
        nc.gpsimd.dma_start(out=wq_b, in_=Wq.rearrange("(c p) e -> p c e", p=P))
        wk_b = consts.tile([P, DC, E], BF16)
        nc.gpsimd.dma_start(out=wk_b, in_=Wk.rearrange("(c p) e -> p c e", p=P))
        wv_b = consts.tile([P, DC, E], BF16)
        nc.gpsimd.dma_start(out=wv_b, in_=Wv.rearrange("(c p) e -> p c e", p=P))

        # biases: per-partition [E, 1] for QT/KT evac; broadcast [P, E] for V
        bq_s = consts.tile([E, 1], F32)
        nc.sync.dma_start(out=bq_s, in_=bq[:, None])
        bk_s = consts.tile([E, 1], F32)
        nc.sync.dma_start(out=bk_s, in_=bk[:, None])
        bv_s = consts.tile([E, 1], F32)
        nc.sync.dma_start(out=bv_s, in_=bv[:, None])

        # mask values per key, [partition = t2 % 128, col = t2 // 128]
        mk = consts.tile([P, NT2], F32)
        nc.sync.dma_start(out=mk, in_=mask.rearrange("(c p) -> p c", p=P))

        qT = persist.tile([P, DC, t1l], BF16)
        kT = persist.tile([P, DC, t2], BF16)
        vT = persist.tile([P, DC, t2], BF16)
        # QT/KT duplicated on partitions 64-127 for row-packed score matmuls
        QT = persist.tile([P, t1l], BF16)
        KT = persist.tile([P, t2], BF16)
        V1 = persist.tile([P, NT2, E + 1], BF16)
        out_sb = persist.tile([P, NT1, E], F32)

        # the "ones" column of V1 carries the mask directly: the masked
        # softmax denominator is sum(mask * exp)
        nc.vector.tensor_copy(out=V1[:, :, E], in_=mk)

        # Stage 1 streams k/v 512-row blocks (load -> transpose -> project)
        # and runs scores/exp/PV for the first two t1-halves on each chunk
        # pair as it becomes ready. Stage 2 finishes the remaining t1-halves
        # from SBUF-resident KT/V1. Score matmul pairs run concurrently in
        # array rows 0-63 / 64-127 (row packing, contract dim is 64) and
        # share one exp call; the mask is folded into V1 rows.
        HW = min(512, t1l)
        NHALF = t1l // HW
        stream_halves = list(range(min(2, NHALF)))
        post_halves = list(range(len(stream_halves), NHALF))
        NP2 = max(1, NT2 // 2)
        pv_tiles = {}
        evac_flip = [0]

        with (
            tc.tile_pool(name="expp", bufs=4) as expp,
            tc.tile_pool(name="ep", bufs=3) as ep,
            tc.tile_pool(name="psPV", bufs=1, space="PSUM") as psPV,
        ):
            def evac_copy(out_ap, in_ap):
                # 2-of-3 on DVE, 1-of-3 on ACT (ACT also carries the exps)
                evac_flip[0] = (evac_flip[0] + 1) % 3
                if evac_flip[0]:
                    nc.vector.tensor_copy(out=out_ap, in_=in_ap)
                else:
                    nc.scalar.copy(out=out_ap, in_=in_ap)

            # software pipeline: PV matmuls for a (quad, half) are emitted
            # only after the NEXT quad's score matmuls, so the in-order PE
            # stream never stalls waiting for the current exp. Scores land
            # in bf16 PSUM: a 4-chunk quad fits 2 banks and one exp call.
            pending = []
            CPQ = min(2, NT2)   # chunks per score-psum tile
            NQ = max(1, NT2 // CPQ)
            QW = CPQ * HW

            def emit_pv(item):
                qi, h, ex = item
                for u in range(CPQ):
                    c = CPQ * qi + u
                    nc.tensor.matmul(
                        pv_tiles[h], V1[:, c, :], ex[:, u * HW:(u + 1) * HW],
                        start=(c == 0), stop=(c == NT2 - 1))

            def scores_exp_pv(psS, qi, h, tag):
                q0 = h * HW
                ps = psS.tile([P, QW], F32, tag=tag, name=f"s_{h}_{qi}")
                for u in range(CPQ):
                    c = CPQ * qi + u
                    rg = E * (u % 2)
                    nc.tensor.matmul(
                        ps[:, u * HW:(u + 1) * HW],
                        KT[rg:rg + E, c * P:(c + 1) * P],
                        QT[rg:rg + E, q0:q0 + HW], start=True, stop=True,
                        tile_position=(rg, 0))
                ex = expp.tile([P, QW], BF16, tag="e", name=f"e_{h}_{qi}")
                nc.scalar.activation(out=ex, in_=ps, func=EXPF, scale=0.125)
                pending.append((qi, h, ex))
                while len(pending) > 1:
                    emit_pv(pending.pop(0))

            def flush_pv():
                while pending:
                    emit_pv(pending.pop(0))

            orr = out.rearrange("(n p) e -> p n e", p=P)

            def epilogue(h, psO):
                pvt = pv_tiles.pop(h)
                q0 = h * HW
                n0, n1 = q0 // P, (q0 + HW) // P
                ov = ep.tile([E + 1, HW], F32, tag="ov", name=f"ov_{h}")
                nc.vector.tensor_copy(out=ov, in_=pvt)
                for j in range(HW // P):
                    po = psO.tile([P, E + 1], F32, tag="o",
                                  name=f"o_{h}_{j}")
                    nc.tensor.transpose(
                        po, ov[:, j * P:(j + 1) * P],
                        ident_f[0:E + 1, 0:E + 1])
                    rec = ep.tile([P, 1], F32, tag="rec",
                                  name=f"rec_{h}_{j}")
                    nc.vector.reciprocal(rec, po[:, E:E + 1])
                    nc.vector.tensor_scalar_mul(
                        out_sb[:, (q0 + j * P) // P, :], po[:, 0:E], rec)
                nc.sync.dma_start(out=orr[:, n0:n1, :],
                                  in_=out_sb[:, n0:n1, :])

            # ---------------- stage 1: stream ----------------
            with (
                tc.tile_pool(name="stage", bufs=8) as stagep,
                tc.tile_pool(name="psA", bufs=2, space="PSUM") as psA,
                tc.tile_pool(name="psAp", bufs=1, space="PSUM") as psAp,
                tc.tile_pool(name="psS1", bufs=1, space="PSUM") as psS1,
            ):
                for h in stream_halves:
                    pv_tiles[h] = psPV.tile([E + 1, HW], F32,
                                            tag=f"pv{h % 2}", name=f"pv_{h}")

                def lt_block(srcr, dst_T, tb, nsub):
                    st = stagep.tile([P, nsub, D], BF16, tag="stage",
                                     name=f"st_{dst_T.tensor.name}_{tb}")
                    nc.gpsimd.dma_start(
                        out=st, in_=srcr[:, tb * nsub:(tb + 1) * nsub, :])
                    grp = 2 if nsub % 2 == 0 else 1
                    for ns0 in range(0, nsub, grp):
                        pst = psA.tile([P, grp * D], BF16, tag="tps",
                                       name=f"tps_{tb}_{ns0}")
                        for g in range(grp):
                            for j in range(DC):
                                nc.tensor.transpose(
                                    pst[:, g * D + j * P:g * D + (j + 1) * P],
                                    st[:, ns0 + g, j * P:(j + 1) * P],
                                    ident_b)
                        t0 = tb * nsub * P + ns0 * P
                        evac_copy(
                            dst_T[:, :, t0:t0 + grp * P].rearrange(
                                "p j (g c) -> p g j c", c=P),
                            pst.rearrange("p (g j c) -> p g j c", j=DC, c=P))

                def proj_block(src_T, w_b, b_s, dst, tb, tb_sz):
                    # write projection to partitions 0:64 and dup to 64:128
                    ps = psAp.tile([E, tb_sz], F32, tag="pproj",
                                   name=f"pp_{dst.tensor.name}_{tb}")
                    for j in range(DC):
                        nc.tensor.matmul(
                            ps, w_b[:, j],
                            src_T[:, j, tb * tb_sz:(tb + 1) * tb_sz],
                            start=(j == 0), stop=(j == DC - 1))
                    sl = slice(tb * tb_sz, (tb + 1) * tb_sz)
                    nc.vector.tensor_scalar_add(dst[0:E, sl], ps, b_s)
                    nc.scalar.activation(
                        out=dst[E:2 * E, sl], in_=ps,
                        func=mybir.ActivationFunctionType.Identity,
                        bias=b_s, scale=1.0)

                # q pipeline (small staging blocks: faster PE rampup)
                TBQ = min(256, t1l)
                qsr = q.rearrange("(n p) d -> p n d", p=P)
                for tb in range(t1l // TBQ):
                    lt_block(qsr, qT, tb, TBQ // P)
                for tb in range(t1l // TBQ):
                    proj_block(qT, wq_b, bq_s, QT, tb, TBQ)

                # k/v stream; per 512-row block: 4 chunks = 2 score pairs
                TBK = min(TB, t2)
                ksr = k.rearrange("(n p) d -> p n d", p=P)
                vsr = v.rearrange("(n p) d -> p n d", p=P)
                cpb = TBK // P           # chunks per block (= one quad)
                qpb = max(1, cpb // CPQ)  # score quads per block

                # scores/exp/PV for block b are spread through block b+1's
                # transpose/projection work: the single-buffered score psum's
                # WAR wait on each exp is then covered by PE filler instead
                # of stalling the in-order PE at the block tail.
                squads = []

                def emit_squad():
                    if squads:
                        qi, h = squads.pop(0)
                        scores_exp_pv(psS1, qi, h, "s1")

                for blk in range(t2 // TBK):
                    lt_block(ksr, kT, blk, cpb)
                    proj_block(kT, wk_b, bk_s, KT, blk, TBK)
                    emit_squad()
                    lt_block(vsr, vT, blk, cpb)
                    emit_squad()
                    # V^T block [E, TBK] like K^T, then PE-transpose each
                    # 128-chunk to natural layout, masking on evacuation
                    psv = psAp.tile([E, TBK], F32, tag="psv",
                                    name=f"psv_{blk}")
                    for j in range(DC):
                        nc.tensor.matmul(
                            psv, wv_b[:, j],
                            vT[:, j, blk * TBK:(blk + 1) * TBK],
                            start=(j == 0), stop=(j == DC - 1))
                    vts = ep.tile([E, TBK], BF16, tag="vts",
                                  name=f"vts_{blk}")
                    nc.vector.tensor_scalar_add(vts, psv, bv_s)
                    emit_squad()
                    for ci in range(cpb):
                        c = blk * cpb + ci
                        pvn = psA.tile([P, E], BF16, tag="tps",
                                       name=f"pvn_{c}")
                        nc.tensor.transpose(
                            pvn, vts[:, ci * P:(ci + 1) * P],
                            ident_b[0:E, 0:E])
                        # fold the key mask into V rows: masked softmax
                        # = sum(mask*exp*V) / sum(mask*exp)
                        nc.vector.tensor_scalar_mul(
                            V1[:, c, 0:E], pvn, mk[:, c:c + 1])
                    emit_squad()
                    for qb in range(qpb):
                        for h in stream_halves:
                            squads.append((blk * qpb + qb, h))
                while squads:
                    emit_squad()

            # ---------------- stage 2: remaining t1-halves ----------------
            with (
                tc.tile_pool(name="psS2", bufs=2, space="PSUM") as psS2,
                tc.tile_pool(name="psO", bufs=1, space="PSUM") as psO,
            ):
                flush_pv()
                pending_stream_epi = list(stream_halves)
                if post_halves and pending_stream_epi:
                    # free pv0 so the first post half can start accumulating
                    epilogue(pending_stream_epi.pop(0), psO)

                def drain_stream_epi():
                    while pending_stream_epi:
                        epilogue(pending_stream_epi.pop(0), psO)

                if not post_halves:
                    drain_stream_epi()
                for h in post_halves:
                    pv_tiles[h] = psPV.tile([E + 1, HW], F32,
                                            tag=f"pv{h % 2}", name=f"pv_{h}")
                    for qi in range(NQ):
                        scores_exp_pv(psS2, qi, h, "s2")
                        if qi >= 1:
                            drain_stream_epi()
                    drain_stream_epi()
                    flush_pv()
                    epilogue(h, psO)


def build_nc(t1l=T1 // 2, t2=T2):
    nc = bacc.Bacc()
    q = nc.declare_dram_parameter("q", [t1l, D], F32, isOutput=False)
    k = nc.declare_dram_parameter("k", [t2, D], F32, isOutput=False)
    v = nc.declare_dram_parameter("v", [t2, D], F32, isOutput=False)
    mask = nc.declare_dram_parameter("mask", [t2], F32, isOutput=False)
    Wq = nc.declare_dram_parameter("Wq", [D, E], F32, isOutput=False)
    bq = nc.declare_dram_parameter("bq", [E], F32, isOutput=False)
    Wk = nc.declare_dram_parameter("Wk", [D, E], F32, isOutput=False)
    bk = nc.declare_dram_parameter("bk", [E], F32, isOutput=False)
    Wv = nc.declare_dram_parameter("Wv", [D, E], F32, isOutput=False)
    bv = nc.declare_dram_parameter("bv", [E], F32, isOutput=False)
    out = nc.declare_dram_parameter("out", [t1l, E], F32, isOutput=True)
    with tile.TileContext(nc) as tc:
        build_body(tc, nc, q[:], k[:], v[:], mask[:], Wq[:], bq[:], Wk[:],
                   bk[:], Wv[:], bv[:], out[:], t1l, t2)
    nc.compile()
    return nc


_NC_CACHE = {}


def _get_nc():
    if "nc" not in _NC_CACHE:
        _NC_CACHE["nc"] = build_nc()
    return _NC_CACHE["nc"]


def make_in_maps(q, k, v, mask, Wq, bq, Wk, bk, Wv, bv):
    t1l = T1 // 2
    shared = {
        "Wq": np.ascontiguousarray(Wq, np.float32),
        "bq": np.ascontiguousarray(bq, np.float32),
        "Wk": np.ascontiguousarray(Wk, np.float32),
        "bk": np.ascontiguousarray(bk, np.float32),
        "Wv": np.ascontiguousarray(Wv, np.float32),
        "bv": np.ascontiguousarray(bv, np.float32),
    }
    in_maps = []
    for c in range(8):
        b, h = divmod(c, 2)
        in_maps.append({
            "q": np.ascontiguousarray(q[b, h * t1l:(h + 1) * t1l], np.float32),
            "k": np.ascontiguousarray(k[b], np.float32),
            "v": np.ascontiguousarray(v[b], np.float32),
            "mask": np.ascontiguousarray(mask[b, 0], np.float32),
            **shared,
        })
    return in_maps


def assemble_out(results):
    t1l = T1 // 2
    out = np.empty((B, T1, E), np.float32)
    for c in range(8):
        b, h = divmod(c, 2)
        out[b, h * t1l:(h + 1) * t1l] = results[c]["out"]
    return out


def run(inputs, trace=False):
    from concourse.bass_utils import run_bass_kernel_spmd
    _install_ntff_hook()
    nc = _get_nc()
    in_maps = make_in_maps(**inputs)
    res = run_bass_kernel_spmd(nc, in_maps, list(range(8)), trace=trace)
    return assemble_out(res.results), res


def kernel(q, k, v, mask, Wq, bq, Wk, bk, Wv, bv):
    out, _ = run(dict(q=q, k=k, v=v, mask=mask, Wq=Wq, bq=bq, Wk=Wk, bk=bk,
                      Wv=Wv, bv=bv))
    return out


# revision 40
# speedup vs baseline: 1.0436x; 1.0192x over previous
"""Bass/Tile TRN2 kernel for nn_AttentionHead: single-head attention with
q/k/v projections (512->64), key mask, softmax over 4096 keys.

Sharding: 8 cores; core c handles batch c//2, query-half c%2 (2048 queries),
with that batch's full k/v replicated. No collectives.

Per-core dataflow (two stages; k/v streaming overlaps attention compute):
  - SWDGE cast-DMA loads q/k/v fp32 -> bf16 staged [t, d] tiles
  - PE transposes 128x128 blocks -> qT/kT/vT in [d, t] layout
  - TensorE projections: QT/KT [e, t] (duplicated on partitions 64-127 for
    row-packed scores); V via V^T then PE transpose -> V1 [t2, 65] where
    column 64 holds the key mask and V rows are pre-multiplied by the mask
    (masked softmax == sum(mask*exp*V) / sum(mask*exp), no -1e9 bias needed)
  - scores: S^T chunks [t2=128, t1=512] = KT_chunk.T @ QT (contract e=64);
    chunk pairs run concurrently in array row groups 0-63/64-127
  - ScalarE: exp(0.125 * S^T), one call per [128, 1024] psum pair
  - PV: O^T[65, t1] += V1_chunk.T @ expS (row 64 = denominator); PV matmuls
    are emitted one quad behind the scores so the in-order PE never stalls
  - epilogue: PE transpose [65,128] blocks, reciprocal + scale on VectorE
"""

import sys
import types

import numpy as np

import concourse.bass as bass
import concourse.tile as tile
from concourse import bacc, mybir
from concourse.masks import make_identity

B, T1, T2, D, E = 4, 4096, 4096, 512, 64
P = 128
F32 = mybir.dt.float32
BF16 = mybir.dt.bfloat16
EXPF = mybir.ActivationFunctionType.Exp
MULT = mybir.AluOpType.mult
ADD = mybir.AluOpType.add


def _install_ntff_hook():
    """Make trace=True usable under axon when antenv.axon_hooks is absent."""
    try:
        import antenv.axon_hooks  # noqa: F401
        return
    except ImportError:
        pass
    try:
        from trn_agent_boot.trn_boot import _ntff_profile_via_ctypes
        hook = _ntff_profile_via_ctypes("/opt/axon/libaxon_pjrt.so")
    except Exception:
        hook = None
    mod = types.ModuleType("antenv.axon_hooks")
    mod.get_axon_ntff_profile_hook = lambda: hook
    mod.set_axon_ntff_profile_hook = lambda h: None
    sys.modules["antenv.axon_hooks"] = mod


def _bcast_ap(ap, parts):
    """Broadcast a 1-D DRAM AP across `parts` partitions (stride-0 DMA)."""
    return bass.AP(tensor=ap.tensor, offset=ap.offset, ap=[[0, parts], ap.ap[0]])


def build_body(tc, nc, q, k, v, mask, Wq, bq, Wk, bk, Wv, bv, out, t1l, t2):
    DC = D // P            # 4 d-chunks
    NT2 = t2 // P          # t2 chunks of 128
    NT1 = t1l // P
    TB = 512               # staging/projection block (t rows)
    T1B = min(1024, t1l)   # phase-B t1 pass width

    with (
        tc.tile_pool(name="consts", bufs=1) as consts,
        tc.tile_pool(name="persist", bufs=1) as persist,
    ):
        ident_b = consts.tile([P, P], BF16)
        make_identity(nc, ident_b)
        ident_f = consts.tile([P, P], F32)
        make_identity(nc, ident_f)

        # weights, bf16, d on partitions: [P, DC, E]
        wq_b = consts.tile([P, DC, E], BF16)
        nc.gpsimd.dma_start(out=wq_b, in_=Wq.rearrange("(c p) e -> p c e", p=P))
        wk_b = consts.tile([P, DC, E], BF16)
        nc.gpsimd.dma_start(out=wk_b, in_=Wk.rearrange("(c p) e -> p c e", p=P))
        wv_b = consts.tile([P, DC, E], BF16)
        nc.gpsimd.dma_start(out=wv_b, in_=Wv.rearrange("(c p) e -> p c e", p=P))

        # biases: per-partition [E, 1] for QT/KT evac; broadcast [P, E] for V
        bq_s = consts.tile([E, 1], F32)
        nc.sync.dma_start(out=bq_s, in_=bq[:, None])
        bk_s = consts.tile([E, 1], F32)
        nc.sync.dma_start(out=bk_s, in_=bk[:, None])
        bv_s = consts.tile([E, 1], F32)
        nc.sync.dma_start(out=bv_s, in_=bv[:, None])

        # mask values per key, [partition = t2 % 128, col = t2 // 128]
        mk = consts.tile([P, NT2], F32)
        nc.sync.dma_start(out=mk, in_=mask.rearrange("(c p) -> p c", p=P))

        qT = persist.tile([P, DC, t1l], BF16)
        kT = persist.tile([P, DC, t2], BF16)
        vT = persist.tile([P, DC, t2], BF16)
        # QT/KT duplicated on partitions 64-127 for row-packed score matmuls
        QT = persist.tile([P, t1l], BF16)
        KT = persist.tile([P, t2], BF16)
        V1 = persist.tile([P, NT2, E + 1], BF16)
        out_sb = persist.tile([P, NT1, E], F32)

        # the "ones" column of V1 carries the mask directly: the masked
        # softmax denominator is sum(mask * exp)
        nc.vector.tensor_copy(out=V1[:, :, E], in_=mk)

        # Stage 1 streams k/v 512-row blocks (load -> transpose -> project)
        # and runs scores/exp/PV for the first two t1-halves on each chunk
        # pair as it becomes ready. Stage 2 finishes the remaining t1-halves
        # from SBUF-resident KT/V1. Score matmul pairs run concurrently in
        # array rows 0-63 / 64-127 (row packing, contract dim is 64) and
        # share one exp call; the mask is folded into V1 rows.
        HW = min(512, t1l)
        NHALF = t1l // HW
        stream_halves = list(range(min(2, NHALF)))
        post_halves = list(range(len(stream_halves), NHALF))
        NP2 = max(1, NT2 // 2)
        pv_tiles = {}
        evac_flip = [0]

        with (
            tc.tile_pool(name="expp", bufs=4) as expp,
            tc.tile_pool(name="ep", bufs=3) as ep,
            tc.tile_pool(name="psPV", bufs=1, space="PSUM") as psPV,
        ):
            def evac_copy(out_ap, in_ap):
                # 2-of-3 on DVE, 1-of-3 on ACT (ACT also carries the exps)
                evac_flip[0] = (evac_flip[0] + 1) % 3
                if evac_flip[0]:
                    nc.vector.tensor_copy(out=out_ap, in_=in_ap)
                else:
                    nc.scalar.copy(out=out_ap, in_=in_ap)

            # software pipeline: PV matmuls for a (quad, half) are emitted
            # only after the NEXT quad's score matmuls, so the in-order PE
            # stream never stalls waiting for the current exp. Scores land
            # in bf16 PSUM: a 4-chunk quad fits 2 banks and one exp call.
            pending = []
            CPQ = min(2, NT2)   # chunks per score-psum tile
            NQ = max(1, NT2 // CPQ)
            QW = CPQ * HW

            def emit_pv(item):
                qi, h, ex = item
                for u in range(CPQ):
                    c = CPQ * qi + u
                    nc.tensor.matmul(
                        pv_tiles[h], V1[:, c, :], ex[:, u * HW:(u + 1) * HW],
                        start=(c == 0), stop=(c == NT2 - 1))

            def scores_exp_pv(psS, qi, h, tag):
                q0 = h * HW
                ps = psS.tile([P, QW], F32, tag=tag, name=f"s_{h}_{qi}")
                for u in range(CPQ):
                    c = CPQ * qi + u
                    rg = E * (u % 2)
                    nc.tensor.matmul(
                        ps[:, u * HW:(u + 1) * HW],
                        KT[rg:rg + E, c * P:(c + 1) * P],
                        QT[rg:rg + E, q0:q0 + HW], start=True, stop=True,
                        tile_position=(rg, 0))
                ex = expp.tile([P, QW], BF16, tag="e", name=f"e_{h}_{qi}")
                nc.scalar.activation(out=ex, in_=ps, func=EXPF, scale=0.125)
                pending.append((qi, h, ex))
                while len(pending) > 1:
                    emit_pv(pending.pop(0))

            def flush_pv():
                while pending:
                    emit_pv(pending.pop(0))

            orr = out.rearrange("(n p) e -> p n e", p=P)

            def epilogue(h, psO):
                pvt = pv_tiles.pop(h)
                q0 = h * HW
                n0, n1 = q0 // P, (q0 + HW) // P
                ov = ep.tile([E + 1, HW], F32, tag="ov", name=f"ov_{h}")
                nc.vector.tensor_copy(out=ov, in_=pvt)
                for j in range(HW // P):
                    po = psO.tile([P, E + 1], F32, tag="o",
                                  name=f"o_{h}_{j}")
                    nc.tensor.transpose(
                        po, ov[:, j * P:(j + 1) * P],
                        ident_f[0:E + 1, 0:E + 1])
                    rec = ep.tile([P, 1], F32, tag="rec",
                                  name=f"rec_{h}_{j}")
                    nc.vector.reciprocal(rec, po[:, E:E + 1])
                    nc.vector.tensor_scalar_mul(
                        out_sb[:, (q0 + j * P) // P, :], po[:, 0:E], rec)
                nc.sync.dma_start(out=orr[:, n0:n1, :],
                                  in_=out_sb[:, n0:n1, :])

            # ---------------- stage 1: stream ----------------
            with (
                tc.tile_pool(name="stage", bufs=8) as stagep,
                tc.tile_pool(name="psA", bufs=2, space="PSUM") as psA,
                tc.tile_pool(name="psAp", bufs=1, space="PSUM") as psAp,
                tc.tile_pool(name="psS1", bufs=1, space="PSUM") as psS1,
            ):
                for h in stream_halves:
                    pv_tiles[h] = psPV.tile([E + 1, HW], F32,
                                            tag=f"pv{h % 2}", name=f"pv_{h}")

                def lt_block(srcr, dst_T, tb, nsub):
                    st = stagep.tile([P, nsub, D], BF16, tag="stage",
                                     name=f"st_{dst_T.tensor.name}_{tb}")
                    nc.gpsimd.dma_start(
                        out=st, in_=srcr[:, tb * nsub:(tb + 1) * nsub, :])
                    grp = 2 if nsub % 2 == 0 else 1
                    for ns0 in range(0, nsub, grp):
                        pst = psA.tile([P, grp * D], BF16, tag="tps",
                                       name=f"tps_{tb}_{ns0}")
                        for g in range(grp):
                            for j in range(DC):
                                nc.tensor.transpose(
                                    pst[:, g * D + j * P:g * D + (j + 1) * P],
                                    st[:, ns0 + g, j * P:(j + 1) * P],
                                    ident_b)
                        t0 = tb * nsub * P + ns0 * P
                        evac_copy(
                            dst_T[:, :, t0:t0 + grp * P].rearrange(
                                "p j (g c) -> p g j c", c=P),
                            pst.rearrange("p (g j c) -> p g j c", j=DC, c=P))

                def proj_block(src_T, w_b, b_s, dst, tb, tb_sz):
                    # write projection to partitions 0:64 and dup to 64:128
                    ps = psAp.tile([E, tb_sz], F32, tag="pproj",
                                   name=f"pp_{dst.tensor.name}_{tb}")
                    for j in range(DC):
                        nc.tensor.matmul(
                            ps, w_b[:, j],
                            src_T[:, j, tb * tb_sz:(tb + 1) * tb_sz],
                            start=(j == 0), stop=(j == DC - 1))
                    sl = slice(tb * tb_sz, (tb + 1) * tb_sz)
                    nc.vector.tensor_scalar_add(dst[0:E, sl], ps, b_s)
                    nc.scalar.activation(
                        out=dst[E:2 * E, sl], in_=ps,
                        func=mybir.ActivationFunctionType.Identity,
                        bias=b_s, scale=1.0)

                # q pipeline (small staging blocks: faster PE rampup)
                TBQ = min(256, t1l)
                qsr = q.rearrange("(n p) d -> p n d", p=P)
                for tb in range(t1l // TBQ):
                    lt_block(qsr, qT, tb, TBQ // P)
                for tb in range(t1l // TBQ):
                    proj_block(qT, wq_b, bq_s, QT, tb, TBQ)

                # k/v stream; per 512-row block: 4 chunks = 2 score pairs
                TBK = min(TB, t2)
                ksr = k.rearrange("(n p) d -> p n d", p=P)
                vsr = v.rearrange("(n p) d -> p n d", p=P)
                cpb = TBK // P           # chunks per block (= one quad)
                for blk in range(t2 // TBK):
                    lt_block(ksr, kT, blk, cpb)
                    proj_block(kT, wk_b, bk_s, KT, blk, TBK)
                    lt_block(vsr, vT, blk, cpb)
                    # V^T block [E, TBK] like K^T, then PE-transpose each
                    # 128-chunk to natural layout, masking on evacuation
                    psv = psAp.tile([E, TBK], F32, tag="psv",
                                    name=f"psv_{blk}")
                    for j in range(DC):
                        nc.tensor.matmul(
                            psv, wv_b[:, j],
                            vT[:, j, blk * TBK:(blk + 1) * TBK],
                            start=(j == 0), stop=(j == DC - 1))
                    vts = ep.tile([E, TBK], BF16, tag="vts",
                                  name=f"vts_{blk}")
                    nc.vector.tensor_scalar_add(vts, psv, bv_s)
                    for ci in range(cpb):
                        c = blk * cpb + ci
                        pvn = psA.tile([P, E], BF16, tag="tps",
                                       name=f"pvn_{c}")
                        nc.tensor.transpose(
                            pvn, vts[:, ci * P:(ci + 1) * P],
                            ident_b[0:E, 0:E])
                        # fold the key mask into V rows: masked softmax
                        # = sum(mask*exp*V) / sum(mask*exp)
                        nc.vector.tensor_scalar_mul(
                            V1[:, c, 0:E], pvn, mk[:, c:c + 1])
                    for qb in range(max(1, cpb // CPQ)):
                        for h in stream_halves:
                            scores_exp_pv(psS1, blk * max(1, cpb // CPQ) + qb,
                                          h, "s1")

            # ---------------- stage 2: remaining t1-halves ----------------
            with (
                tc.tile_pool(name="psS2", bufs=2, space="PSUM") as psS2,
                tc.tile_pool(name="psO", bufs=1, space="PSUM") as psO,
            ):
                flush_pv()
                pending_stream_epi = list(stream_halves)
                if post_halves and pending_stream_epi:
                    # free pv0 so the first post half can start accumulating
                    epilogue(pending_stream_epi.pop(0), psO)

                def drain_stream_epi():
                    while pending_stream_epi:
                        epilogue(pending_stream_epi.pop(0), psO)

                if not post_halves:
                    drain_stream_epi()
                for h in post_halves:
                    pv_tiles[h] = psPV.tile([E + 1, HW], F32,
                                            tag=f"pv{h % 2}", name=f"pv_{h}")
                    for qi in range(NQ):
                        scores_exp_pv(psS2, qi, h, "s2")
                        if qi >= 1:
                            drain_stream_epi()
                    drain_stream_epi()
                    flush_pv()
                    epilogue(h, psO)


def build_nc(t1l=T1 // 2, t2=T2):
    nc = bacc.Bacc()
    q = nc.declare_dram_parameter("q", [t1l, D], F32, isOutput=False)
    k = nc.declare_dram_parameter("k", [t2, D], F32, isOutput=False)
    v = nc.declare_dram_parameter("v", [t2, D], F32, isOutput=False)
    mask = nc.declare_dram_parameter("mask", [t2], F32, isOutput=False)
    Wq = nc.declare_dram_parameter("Wq", [D, E], F32, isOutput=False)
    bq = nc.declare_dram_parameter("bq", [E], F32, isOutput=False)
    Wk = nc.declare_dram_parameter("Wk", [D, E], F32, isOutput=False)
    bk = nc.declare_dram_parameter("bk", [E], F32, isOutput=False)
    Wv = nc.declare_dram_parameter("Wv", [D, E], F32, isOutput=False)
    bv = nc.declare_dram_parameter("bv", [E], F32, isOutput=False)
    out = nc.declare_dram_parameter("out", [t1l, E], F32, isOutput=True)
    with tile.TileContext(nc) as tc:
        build_body(tc, nc, q[:], k[:], v[:], mask[:], Wq[:], bq[:], Wk[:],
                   bk[:], Wv[:], bv[:], out[:], t1l, t2)
    nc.compile()
    return nc


_NC_CACHE = {}


def _get_nc():
    if "nc" not in _NC_CACHE:
        _NC_CACHE["nc"] = build_nc()
    return _NC_CACHE["nc"]


def make_in_maps(q, k, v, mask, Wq, bq, Wk, bk, Wv, bv):
    t1l = T1 // 2
    shared = {
        "Wq": np.ascontiguousarray(Wq, np.float32),
        "bq": np.ascontiguousarray(bq, np.float32),
        "Wk": np.ascontiguousarray(Wk, np.float32),
        "bk": np.ascontiguousarray(bk, np.float32),
        "Wv": np.ascontiguousarray(Wv, np.float32),
        "bv": np.ascontiguousarray(bv, np.float32),
    }
    in_maps = []
    for c in range(8):
        b, h = divmod(c, 2)
        in_maps.append({
            "q": np.ascontiguousarray(q[b, h * t1l:(h + 1) * t1l], np.float32),
            "k": np.ascontiguousarray(k[b], np.float32),
            "v": np.ascontiguousarray(v[b], np.float32),
            "mask": np.ascontiguousarray(mask[b, 0], np.float32),
            **shared,
        })
    return in_maps


def assemble_out(results):
    t1l = T1 // 2
    out = np.empty((B, T1, E), np.float32)
    for c in range(8):
        b, h = divmod(c, 2)
        out[b, h * t1l:(h + 1) * t1l] = results[c]["out"]
    return out


def run(inputs, trace=False):
    from concourse.bass_utils import run_bass_kernel_spmd
    _install_ntff_hook()
    nc = _get_nc()
    in_maps = make_in_maps(**inputs)
    res = run_bass_kernel_spmd(nc, in_maps, list(range(8)), trace=trace)
    return assemble_out(res.results), res


def kernel(q, k, v, mask, Wq, bq, Wk, bk, Wv, bv):
    out, _ = run(dict(q=q, k=k, v=v, mask=mask, Wq=Wq, bq=bq, Wk=Wk, bk=bk,
                      Wv=Wv, bv=bv))
    return out
